# revision 18
# baseline (speedup 1.0000x reference)
"""BERT self-attention on 8 Trainium2 NeuronCores — v2.

Sharding: data-parallel over batch (B=8 -> one batch element per core).

Changes vs v1 (263.5us baseline):
  - All matmul operands bf16 (host converts x/W): halves input DMA
    (PE was idle ~17us at start waiting on f32r loads), LDWEIGHTS
    drops 330ns -> 95ns, and lifts the f32r N>=256 restriction.
  - K/Q projections are interleaved into the attention pair loop
    (pair hp's stream also computes pair hp+1's K/Q), so the PE has
    ~2.35us/kc of work vs ACT's 2.08us/kc of exp — the v1 attention
    phase was ACT-bound with the PE stalling ~1us per kc.
  - Softmax denominators divide on GPSIMD (tensor_tensor divide)
    instead of DVE reciprocal (6.5us per single-lane [1,1024]!) or
    ACT Ln+Exp (which competed with the exp stream).
  - No bias matmuls in V-proj/out-proj inner loops: bk cancels in
    softmax, bq is added during Q's PSUM evacuation, and bv/bo fold
    into one precomputed row b' = bo + Wo@bv added via DVE during
    the output evacuation (P rows sum to 1 after normalize, so
    ctx_norm needs +bv, and (ctx_norm+bv)@Wo^T = ctx_norm@Wo^T + b').
  - Attention layout unchanged: ST[k,q] = K Q^T per head so softmax's
    reduction lands on partitions; exp via ScalarE with mask as
    per-partition bias; denominator from a ones column in V (row 64
    of ctx^T); P^T V accumulates ctx^T [d, q].
"""

import numpy as np
import ml_dtypes

import concourse.bass as bass  # noqa: F401
import concourse.mybir as mybir
import concourse.tile as tile
from concourse import bacc
from concourse.bass_interp import get_hw_module
from concourse.bass_utils import run_bass_kernel_spmd

B, L, H = 8, 1024, 768
NH, HD = 12, 64
NC = H // 128          # 6 chunks of hidden dim
LC = L // 128          # 8 chunks of sequence dim
NP = NH // 2           # 6 head pairs
F32 = mybir.dt.float32
F32R = mybir.dt.float32r
BF = mybir.dt.bfloat16
EXP = mybir.ActivationFunctionType.Exp

# normalize ctx by the softmax denominator via GPSIMD divide (True) or
# ACT exp(-ln d) broadcast-multiply (False, v1-style fallback)
USE_POOL_DIV = True


def build_bass(compute_rounded: bool = True):
    nc = bacc.Bacc("TRN2", debug=False, num_devices=8)

    # Pin Exp/Ln to the combined table set so walrus never alternates
    # ACT table loads mid-kernel (one ~1.3us load total).
    from concourse.hw_specs import get_activation_tables

    _tabs = get_activation_tables(nc.m.arch)
    _E = mybir.ActivationFunctionType.Exp
    _L = mybir.ActivationFunctionType.Ln
    if "natural_log_exp_and_others" in _tabs:
        for _name, _fns in _tabs.items():
            if _name != "natural_log_exp_and_others":
                _fns.discard(_E)
                _fns.discard(_L)

    xt_e = nc.declare_dram_parameter("xt", [H, L], BF, isOutput=False)
    wqt_e = nc.declare_dram_parameter("wqt", [H, H], BF, isOutput=False)
    wkt_e = nc.declare_dram_parameter("wkt", [H, H], BF, isOutput=False)
    wvt_e = nc.declare_dram_parameter("wvt", [H, H], BF, isOutput=False)
    wot_e = nc.declare_dram_parameter("wot", [H, H], BF, isOutput=False)
    bq_e = nc.declare_dram_parameter("bq", [H], F32, isOutput=False)
    bk_e = nc.declare_dram_parameter("bk", [H], F32, isOutput=False)
    bv_e = nc.declare_dram_parameter("bv", [H], BF, isOutput=False)
    bo_e = nc.declare_dram_parameter("bo", [H], BF, isOutput=False)
    mask_e = nc.declare_dram_parameter("mask", [L], F32, isOutput=False)
    out_e = nc.declare_dram_parameter("out", [L, H], F32, isOutput=True)

    with tile.TileContext(nc) as tc:
        with (
            tc.tile_pool(name="small", bufs=1) as small,
            tc.tile_pool(name="acts", bufs=1) as acts,
            tc.tile_pool(name="w", bufs=1) as wpool,
            tc.tile_pool(name="outp", bufs=2) as out_pool,
            tc.tile_pool(name="norm", bufs=1) as norm_pool,
            tc.tile_pool(name="eta", bufs=4) as eta_pool,
            tc.tile_pool(name="etb", bufs=9) as etb_pool,
            tc.tile_pool(name="psA", bufs=3, space="PSUM") as psA,
            tc.tile_pool(name="psB", bufs=1, space="PSUM") as psB,
        ):
            # ---- persistent SBUF tensors ----
            # xt/wv as per-chunk tiles: DMA-write dependencies resolve per
            # tile, so the V projection starts on chunk 0 instead of
            # waiting ~9us for the full 2.6MB to land
            xt_cs = [acts.tile([128, L], BF, name=f"xt{c}") for c in range(NC)]
            wv_cs = [acts.tile([128, H], BF, name=f"wv{c}") for c in range(NC)]
            qt_sb = acts.tile([128, NC, L], BF)
            kt_sb = acts.tile([128, NH, L], BF)  # per-head K^T, other 64 rows 0
            nc.gpsimd.memset(kt_sb[:], 0.0)
            v_sb = acts.tile([128, LC, NH, HD + 1], BF)  # [..., 64] = ones col
            ctxt_sb = acts.tile([128, NC, L], BF)
            wk_sb = wpool.tile([128, NC, H], BF)
            wq_sb = wpool.tile([128, NC, H], BF)
            wo_sb = wpool.tile([128, NC, H], BF)

            ones32 = small.tile([128, 128], F32)
            nc.vector.memset(ones32[:], 1.0)
            # warm the PE p-state during the input-DMA wait (~6us of
            # framework preamble + DGE latency before the first byte
            # lands): the PE runs at half clock for its first ~3us of
            # work after any idle period, which would otherwise be paid
            # by the V projection's first groups
            warm = psA.tile([64, 128], F32, tag="psA")
            for _ in range(56):
                nc.tensor.matmul(
                    warm[:, :], ones32[0:1, 0:64], ones32[0:1, :],
                    start=True, stop=True,
                )
            ones_bf = small.tile([1, 128], BF)
            nc.vector.tensor_copy(ones_bf[:], ones32[0:1, :])
            ones_r = small.tile([128, 64], F32R)
            nc.vector.tensor_copy(ones_r[:], ones32[:, 0:64])
            nc.vector.tensor_copy(
                v_sb[:, :, :, HD],
                ones32[:, 0 : LC * NH].rearrange("p (a b) -> p a b", a=LC),
            )

            # ---- input DMA: one descriptor-set per tensor (per-chunk
            # dispatch serialized ~0.6us each on the SP sequencer and
            # delayed compute start by ~6us). xt first (V-proj gates on
            # all of it), then wv/wk/wq/wo in consumption order; small
            # tensors go out on the DVE queue in parallel. ----
            for c in range(NC):
                nc.sync.dma_start(
                    xt_cs[c][:],
                    xt_e[:].rearrange("(c p) q -> p c q", p=128)[:, c, :],
                )
                nc.sync.dma_start(
                    wv_cs[c][:],
                    wvt_e[:].rearrange("(c p) d -> p c d", p=128)[:, c, :],
                )
            nc.sync.dma_start(
                wk_sb[:], wkt_e[:].rearrange("(c p) d -> p c d", p=128)
            )
            nc.sync.dma_start(
                wq_sb[:], wqt_e[:].rearrange("(c p) d -> p c d", p=128)
            )
            mask_sb = small.tile([128, LC], F32)
            nc.scalar.dma_start(mask_sb[:], mask_e[:].rearrange("(c p) -> p c", p=128))
            bq_sb = small.tile([128, NC], F32)
            nc.scalar.dma_start(bq_sb[:], bq_e[:].rearrange("(c p) -> p c", p=128))
            bk_sb = small.tile([128, NC], F32)
            nc.scalar.dma_start(bk_sb[:], bk_e[:].rearrange("(c p) -> p c", p=128))
            bv_sb = small.tile([128, NC], BF)
            nc.scalar.dma_start(bv_sb[:], bv_e[:].rearrange("(c p) -> p c", p=128))
            bo_sb = small.tile([1, H], BF)
            nc.scalar.dma_start(bo_sb[:], bo_e[None, :])
            nc.sync.dma_start(
                wo_sb[:], wot_e[:].rearrange("(c p) d -> p c d", p=128)
            )

            # =========== V projection: v[q, h, d] = x Wv^T ===========
            for lc in range(LC):
                ps = psA.tile([128, 1024], F32, tag="psA")
                for off, width in ((0, 512), (512, 256)):
                    for kc in range(NC):
                        nc.tensor.matmul(
                            ps[:, off : off + width],
                            xt_cs[kc][:, lc * 128 : lc * 128 + 128],
                            wv_cs[kc][:, off : off + width],
                            start=(kc == 0),
                            stop=(kc == NC - 1),
                        )
                nc.scalar.copy(
                    v_sb[:, lc, :, 0:HD],
                    ps[:, 0:H].rearrange("p (h d) -> p h d", d=HD),
                )

            # ---- K^T / Q^T projection emitters: out[d, q] = W x^T.
            # Emitted as 6-matmul half-projections (one qh each) so they
            # interleave into the attention pair loop as short PE bursts
            # the ACT exp stream can ride through. ----
            def emit_kproj_half(dc, qh):
                o = qh * 512
                ps = psA.tile([128, 512], F32, tag="psA")
                for kc in range(NC):
                    nc.tensor.matmul(
                        ps[:, 0:512],
                        wk_sb[:, kc, dc * 128 : dc * 128 + 128],
                        xt_cs[kc][:, o : o + 512],
                        start=(kc == 0),
                        stop=(kc == NC - 1),
                    )
                # split heads: even head -> rows 0:64 of slot 2dc, odd ->
                # rows 64:128 of slot 2dc+1 (rest stays zero-padded).
                # bk is added but cancels in softmax anyway; the add doubles
                # as the PSUM evacuation copy.
                nc.vector.tensor_scalar_add(
                    kt_sb[0:64, 2 * dc, o : o + 512], ps[0:64, 0:512],
                    bk_sb[0:64, dc : dc + 1],
                )
                nc.vector.tensor_scalar_add(
                    kt_sb[64:128, 2 * dc + 1, o : o + 512], ps[64:128, 0:512],
                    bk_sb[64:128, dc : dc + 1],
                )

            def emit_qproj_half(dc, qh):
                o = qh * 512
                ps = psA.tile([128, 512], F32, tag="psA")
                for kc in range(NC):
                    nc.tensor.matmul(
                        ps[:, 0:512],
                        wq_sb[:, kc, dc * 128 : dc * 128 + 128],
                        xt_cs[kc][:, o : o + 512],
                        start=(kc == 0),
                        stop=(kc == NC - 1),
                    )
                nc.vector.tensor_scalar_add(
                    qt_sb[:, dc, o : o + 512], ps[:, 0:512], bq_sb[:, dc : dc + 1]
                )

            for qh in range(2):
                emit_kproj_half(0, qh)
            for qh in range(2):
                emit_qproj_half(0, qh)

            # =========== attention: pair hp, with KQ proj of hp+1 ==========
            LOG = mybir.ActivationFunctionType.Ln

            def make_normalize(hp, ctxu_a, ctxu_b, last=False):
                rrows = [None, None]

                def chain_head(idx, ctxu, which):
                    # reciprocal chain — DMA/DVE/ACT only, NO PE
                    # instructions, so it can be emitted right after each
                    # evacuation and resolve its latency while the PE queue
                    # streams unrelated work. (PE-side consumers emitted
                    # here would head-of-line-block the in-order PE queue.)
                    if not last:
                        # SBUF->SBUF DMA the denominator row into [128,8]
                        # so the DVE reciprocal runs on all 128 lanes (a
                        # [1,1024] reciprocal is single-lane, 6.5us), cast
                        # to f32r cheaply while [128,8], DMA back to a row
                        rt = norm_pool.tile([128, 8], F32, tag="rt" + which)
                        nc.sync.dma_start(rt[:, :], ctxu[64:65, :])
                        rtr = norm_pool.tile([128, 8], F32, tag="rr" + which)
                        nc.vector.reciprocal(rtr[:], rt[:])
                        rtr_r = norm_pool.tile([128, 8], F32R, tag="rc" + which)
                        nc.vector.tensor_copy(rtr_r[:], rtr[:])
                        rrow = norm_pool.tile([1, 1024], F32R, tag="rw" + which)
                        nc.sync.dma_start(rrow[0:1, :], rtr_r[:, :])
                        rrows[idx] = rrow
                    else:
                        # last pair: the out projection gates on this, so
                        # use the short-latency ACT path per head, 1/d =
                        # exp(-ln d) straight from the evacuated row (ACT
                        # is idle by now; head a's chain runs during head
                        # b's PV sweep)
                        lnd = norm_pool.tile([1, 1024], F32, tag="ln" + which)
                        nc.scalar.activation(lnd[:], ctxu[64:65, :], LOG)
                        rrow = norm_pool.tile([1, 1024], F32R, tag="rw" + which)
                        nc.scalar.activation(rrow[:], lnd[:], EXP, scale=-1.0)
                        rrows[idx] = rrow

                def emit_finish(bc_pool):
                    for idx, (ctxu, which) in (
                        (0, (ctxu_a, "a")), (1, (ctxu_b, "b"))
                    ):
                        # broadcast 1/d over 64 partitions via a 1-row f32r
                        # matmul, then scale ctx^T. Even head multiplies on
                        # DVE reading bc straight from PSUM; odd head goes
                        # through Pool mid-kernel (parallel with DVE) but
                        # through DVE on the latency-critical last pair,
                        # then lifts to partitions 64:128 via DMA.
                        bc = bc_pool.tile(
                            [64, 1024], F32,
                            tag="psA" if bc_pool is psA else "psB",
                        )
                        for o in (0, 512):
                            nc.tensor.matmul(
                                bc[:, o : o + 512],
                                ones_r[0:1, 0:64],
                                rrows[idx][0:1, o : o + 512],
                                start=True,
                                stop=True,
                            )
                        if which == "a":
                            nc.vector.tensor_tensor(
                                ctxt_sb[0:64, hp, :], ctxu[0:64, :], bc[:],
                                mybir.AluOpType.mult,
                            )
                        else:
                            tmp_o = norm_pool.tile([64, 1024], BF, tag="tmp")
                            if last:
                                nc.vector.tensor_tensor(
                                    tmp_o[:], ctxu[0:64, :], bc[:],
                                    mybir.AluOpType.mult,
                                )
                            else:
                                bc_sb = norm_pool.tile(
                                    [64, 1024], F32, tag="bc" + which
                                )
                                nc.vector.tensor_copy(bc_sb[:], bc[:])
                                nc.gpsimd.tensor_tensor(
                                    tmp_o[:], ctxu[0:64, :], bc_sb[:],
                                    mybir.AluOpType.mult,
                                )
                            nc.sync.dma_start(ctxt_sb[64:128, hp, :], tmp_o[:])

                return chain_head, emit_finish

            # b' = bo + Wo @ bv (P rows sum to 1 after normalize, so bv
            # contributes bv @ Wo^T, a constant row added to every output
            # q). Emitted during pair NP-2 so its PE matmuls and the
            # single-lane DVE copy hide inside the attention stream
            # instead of gating the output projection.
            bbc_holder = [None]

            def emit_bprime():
                bps = psA.tile([1, 1024], F32, tag="psA")
                for off, width in ((0, 512), (512, 256)):
                    for c in range(NC):
                        nc.tensor.matmul(
                            bps[0:1, off : off + width],
                            bv_sb[:, c : c + 1],
                            wo_sb[:, c, off : off + width],
                            start=(c == 0),
                            stop=False,
                        )
                    nc.tensor.matmul(
                        bps[0:1, off : off + width],
                        ones_bf[0:1, 0:1],
                        bo_sb[0:1, off : off + width],
                        start=False,
                        stop=True,
                    )
                bexp_sb = small.tile([1, H], BF)
                nc.vector.tensor_copy(bexp_sb[:], bps[0:1, 0:H])
                bbc_ps = psA.tile([128, 1024], F32, tag="psA")
                for off, width in ((0, 512), (512, 256)):
                    nc.tensor.matmul(
                        bbc_ps[:, off : off + width],
                        ones_bf[0:1, 0:128],
                        bexp_sb[0:1, off : off + width],
                        start=True,
                        stop=True,
                    )
                bbc_sb = small.tile([128, H], F32)
                nc.vector.tensor_copy(bbc_sb[:], bbc_ps[:, 0:H])
                bbc_holder[0] = bbc_sb

            # Per pair, head a's PV accumulates during the kc loop and head
            # b's PV runs as a second sweep afterwards, so only ONE ctx
            # PSUM tile is ever live (psB bufs=1, 2 banks) — which buys
            # psA THREE [128,1024] ST slabs (6 banks). With only two, each
            # ST waited on the exp of the previous kc through a full
            # semaphore round-trip: the engines ran in lockstep with ~1us
            # of dead time per kc (87us total in the v5 trace).
            pending_norm = None
            for hp in range(NP):
                ha, hb = 2 * hp, 2 * hp + 1

                def emit_pv(kc, et, h, ctx):
                    first, last = kc == 0, kc == LC - 1
                    for qh in range(2):
                        o = qh * 512
                        nc.tensor.matmul(
                            ctx[0 : HD + 1, o : o + 512],
                            v_sb[:, kc, h, :],
                            et[:, o : o + 512],
                            start=first,
                            stop=last,
                        )

                # rows 0:64 = ctx^T, row 64 = softmax denominator
                ctx_a = psB.tile([128, 1024], F32, tag="psB")
                pv_q = []  # head-a PV at lag 2
                et_bs = []  # head-b et tiles, consumed in the second sweep
                for kc in range(LC):
                    if kc == 5 and pending_norm is not None:
                        pending_norm(psA)
                        pending_norm = None
                    st_a = psA.tile([128, 1024], F32, tag="psA")
                    st_b = psA.tile([128, 1024], F32, tag="psA")
                    for st, h in ((st_a, ha), (st_b, hb)):
                        for qh in range(2):
                            o = qh * 512
                            nc.tensor.matmul(
                                st[:, o : o + 512],
                                kt_sb[:, h, kc * 128 : kc * 128 + 128],
                                qt_sb[:, hp, o : o + 512],
                                start=True,
                                stop=True,
                            )
                    # P^T = exp(S^T/8 + mask_k)
                    et_a = eta_pool.tile([128, 1024], BF, tag="eta")
                    et_b = etb_pool.tile([128, 1024], BF, tag="etb")
                    nc.scalar.activation(
                        et_a[:], st_a[:], EXP,
                        bias=mask_sb[:, kc : kc + 1], scale=0.125,
                    )
                    nc.scalar.activation(
                        et_b[:], st_b[:], EXP,
                        bias=mask_sb[:, kc : kc + 1], scale=0.125,
                    )
                    pv_q.append((kc, et_a))
                    et_bs.append(et_b)
                    if len(pv_q) > 2:
                        kc_, et_ = pv_q.pop(0)
                        emit_pv(kc_, et_, ha, ctx_a)
                    # next pair's K/Q projection, spread as 6-matmul bursts
                    # emitted after this kc's PV so the PE always has
                    # slack-filling work between STs
                    if hp + 1 < NP:
                        if kc == 1:
                            emit_kproj_half(hp + 1, 0)
                        elif kc == 3:
                            emit_kproj_half(hp + 1, 1)
                        elif kc == 4:
                            emit_qproj_half(hp + 1, 0)
                while pv_q:
                    kc_, et_ = pv_q.pop(0)
                    emit_pv(kc_, et_, ha, ctx_a)
                # evacuate head a immediately: frees the single psB slab
                # for head b's sweep
                ctxu_a = norm_pool.tile([65, 1024], F32, tag="cua")
                nc.scalar.copy(ctxu_a[:], ctx_a[0:65, :])
                ctxu_b = norm_pool.tile([65, 1024], F32, tag="cub")
                chain_head, finish = make_normalize(
                    hp, ctxu_a, ctxu_b, last=(hp == NP - 1)
                )
                # head a's reciprocal chain resolves during head b's sweep
                chain_head(0, ctxu_a, "a")
                # fill the evacuation latency with the last projection burst
                if hp + 1 < NP:
                    emit_qproj_half(hp + 1, 1)
                ctx_b = psB.tile([128, 1024], F32, tag="psB")
                for kc in range(LC):
                    emit_pv(kc, et_bs[kc], hb, ctx_b)
                nc.vector.tensor_copy(ctxu_b[:], ctx_b[0:65, :])
                chain_head(1, ctxu_b, "b")
                if hp == NP - 2:
                    emit_bprime()
                pending_norm = finish
            last_norm = pending_norm
            pending_norm = None
            bbc_sb = bbc_holder[0]

            # ---- output projection: out[q, o] = ctx_norm Wo^T + b'.
            # Chunks 0..4 of the first two groups are emitted BEFORE the
            # last pair's normalize matmuls so the PE streams them while
            # that pair's reciprocal chain resolves; chunk 5 (written by
            # the last normalize) closes each accumulation group. ----
            def outproj_front(ps, lc):
                for off, width in ((0, 512), (512, 256)):
                    for c in range(NC - 1):
                        nc.tensor.matmul(
                            ps[:, off : off + width],
                            ctxt_sb[:, c, lc * 128 : lc * 128 + 128],
                            wo_sb[:, c, off : off + width],
                            start=(c == 0),
                            stop=False,
                        )

            def outproj_close(ps, lc):
                c = NC - 1
                for off, width in ((0, 512), (512, 256)):
                    nc.tensor.matmul(
                        ps[:, off : off + width],
                        ctxt_sb[:, c, lc * 128 : lc * 128 + 128],
                        wo_sb[:, c, off : off + width],
                        start=False,
                        stop=True,
                    )
                o_sb = out_pool.tile([128, H], F32, tag="outp")
                nc.vector.tensor_tensor(
                    o_sb[:], ps[:, 0:H], bbc_sb[:], mybir.AluOpType.add
                )
                nc.sync.dma_start(out_e[lc * 128 : lc * 128 + 128, :], o_sb[:])

            ps0 = psA.tile([128, 1024], F32, tag="psA")
            outproj_front(ps0, 0)
            ps1 = psA.tile([128, 1024], F32, tag="psA")
            outproj_front(ps1, 1)
            last_norm(psB)  # bc tiles from the (now free) psB slab
            outproj_close(ps0, 0)
            outproj_close(ps1, 1)
            for lc in range(2, LC):
                ps = psA.tile([128, 1024], F32, tag="psA")
                outproj_front(ps, lc)
                outproj_close(ps, lc)

    nc.finalize()
    nc.m = get_hw_module(nc.m)
    return nc


_NC_CACHE = {}


def _get_nc(compute_rounded: bool = True):
    if compute_rounded not in _NC_CACHE:
        _NC_CACHE[compute_rounded] = build_bass(compute_rounded)
    return _NC_CACHE[compute_rounded]


def make_in_maps(inputs):
    f32 = lambda a: np.ascontiguousarray(np.asarray(a, dtype=np.float32))  # noqa: E731
    bf = lambda a: np.ascontiguousarray(  # noqa: E731
        np.asarray(a, dtype=np.float32).astype(ml_dtypes.bfloat16)
    )
    hs = np.asarray(inputs["hidden_states"], dtype=np.float32)
    mask = f32(inputs["attention_mask"]).reshape(B, L)
    shared = {
        "wqt": bf(np.asarray(inputs["Wq"]).T),
        "wkt": bf(np.asarray(inputs["Wk"]).T),
        "wvt": bf(np.asarray(inputs["Wv"]).T),
        "wot": bf(np.asarray(inputs["Wo"]).T),
        "bq": f32(inputs["bq"]),
        "bk": f32(inputs["bk"]),
        "bv": bf(inputs["bv"]),
        "bo": bf(inputs["bo"]),
    }
    return [
        {"xt": bf(hs[b].T), "mask": mask[b], **shared}
        for b in range(B)
    ]


def run_spmd(inputs, trace=False, compute_rounded=True):
    nc = _get_nc(compute_rounded)
    res = run_bass_kernel_spmd(nc, make_in_maps(inputs), list(range(B)), trace=trace)
    out = np.stack([res.results[b]["out"] for b in range(B)]).astype(np.float32)
    return out, res


def kernel(**inputs) -> np.ndarray:
    out, _ = run_spmd(inputs, trace=False)
    return out


# revision 19
# speedup vs baseline: 1.0695x; 1.0695x over previous
"""BERT self-attention on 8 Trainium2 NeuronCores — v2.

Sharding: data-parallel over batch (B=8 -> one batch element per core).

Changes vs v1 (263.5us baseline):
  - All matmul operands bf16 (host converts x/W): halves input DMA
    (PE was idle ~17us at start waiting on f32r loads), LDWEIGHTS
    drops 330ns -> 95ns, and lifts the f32r N>=256 restriction.
  - K/Q projections are interleaved into the attention pair loop
    (pair hp's stream also computes pair hp+1's K/Q), so the PE has
    ~2.35us/kc of work vs ACT's 2.08us/kc of exp — the v1 attention
    phase was ACT-bound with the PE stalling ~1us per kc.
  - Softmax denominators divide on GPSIMD (tensor_tensor divide)
    instead of DVE reciprocal (6.5us per single-lane [1,1024]!) or
    ACT Ln+Exp (which competed with the exp stream).
  - No bias matmuls in V-proj/out-proj inner loops: bk cancels in
    softmax, bq is added during Q's PSUM evacuation, and bv/bo fold
    into one precomputed row b' = bo + Wo@bv added via DVE during
    the output evacuation (P rows sum to 1 after normalize, so
    ctx_norm needs +bv, and (ctx_norm+bv)@Wo^T = ctx_norm@Wo^T + b').
  - Attention layout unchanged: ST[k,q] = K Q^T per head so softmax's
    reduction lands on partitions; exp via ScalarE with mask as
    per-partition bias; denominator from a ones column in V (row 64
    of ctx^T); P^T V accumulates ctx^T [d, q].
"""

import numpy as np
import ml_dtypes

import concourse.bass as bass  # noqa: F401
import concourse.mybir as mybir
import concourse.tile as tile
from concourse import bacc
from concourse.bass_interp import get_hw_module
from concourse.bass_utils import run_bass_kernel_spmd

B, L, H = 8, 1024, 768
NH, HD = 12, 64
NC = H // 128          # 6 chunks of hidden dim
LC = L // 128          # 8 chunks of sequence dim
NP = NH // 2           # 6 head pairs
F32 = mybir.dt.float32
F32R = mybir.dt.float32r
BF = mybir.dt.bfloat16
EXP = mybir.ActivationFunctionType.Exp

# normalize ctx by the softmax denominator via GPSIMD divide (True) or
# ACT exp(-ln d) broadcast-multiply (False, v1-style fallback)
USE_POOL_DIV = True


def build_bass(compute_rounded: bool = True):
    nc = bacc.Bacc("TRN2", debug=False, num_devices=8)

    # Pin Exp/Ln to the combined table set so walrus never alternates
    # ACT table loads mid-kernel (one ~1.3us load total).
    from concourse.hw_specs import get_activation_tables

    _tabs = get_activation_tables(nc.m.arch)
    _E = mybir.ActivationFunctionType.Exp
    _L = mybir.ActivationFunctionType.Ln
    if "natural_log_exp_and_others" in _tabs:
        for _name, _fns in _tabs.items():
            if _name != "natural_log_exp_and_others":
                _fns.discard(_E)
                _fns.discard(_L)

    xt_e = nc.declare_dram_parameter("xt", [H, L], BF, isOutput=False)
    wqt_e = nc.declare_dram_parameter("wqt", [H, H], BF, isOutput=False)
    wkt_e = nc.declare_dram_parameter("wkt", [H, H], BF, isOutput=False)
    wvt_e = nc.declare_dram_parameter("wvt", [H, H], BF, isOutput=False)
    wot_e = nc.declare_dram_parameter("wot", [H, H], BF, isOutput=False)
    bq_e = nc.declare_dram_parameter("bq", [H], F32, isOutput=False)
    bk_e = nc.declare_dram_parameter("bk", [H], F32, isOutput=False)
    bv_e = nc.declare_dram_parameter("bv", [H], BF, isOutput=False)
    bo_e = nc.declare_dram_parameter("bo", [H], BF, isOutput=False)
    mask_e = nc.declare_dram_parameter("mask", [L], F32, isOutput=False)
    out_e = nc.declare_dram_parameter("out", [L, H], F32, isOutput=True)

    with tile.TileContext(nc) as tc:
        with (
            tc.tile_pool(name="small", bufs=1) as small,
            tc.tile_pool(name="acts", bufs=1) as acts,
            tc.tile_pool(name="w", bufs=1) as wpool,
            tc.tile_pool(name="outp", bufs=2) as out_pool,
            tc.tile_pool(name="norm", bufs=1) as norm_pool,
            tc.tile_pool(name="eta", bufs=4) as eta_pool,
            tc.tile_pool(name="etb", bufs=9) as etb_pool,
            tc.tile_pool(name="psA", bufs=3, space="PSUM") as psA,
            tc.tile_pool(name="psB", bufs=1, space="PSUM") as psB,
        ):
            # ---- persistent SBUF tensors ----
            # xt/wv as per-chunk tiles: DMA-write dependencies resolve per
            # tile, so the V projection starts on chunk 0 instead of
            # waiting ~9us for the full 2.6MB to land
            xt_cs = [acts.tile([128, L], BF, name=f"xt{c}") for c in range(NC)]
            wv_cs = [acts.tile([128, H], BF, name=f"wv{c}") for c in range(NC)]
            qt_sb = acts.tile([128, NC, L], BF)
            kt_sb = acts.tile([128, NH, L], BF)  # per-head K^T, other 64 rows 0
            nc.gpsimd.memset(kt_sb[:], 0.0)
            v_sb = acts.tile([128, LC, NH, HD + 1], BF)  # [..., 64] = ones col
            ctxt_sb = acts.tile([128, NC, L], BF)
            wk_sb = wpool.tile([128, NC, H], BF)
            wq_sb = wpool.tile([128, NC, H], BF)
            wo_sb = wpool.tile([128, NC, H], BF)

            ones32 = small.tile([128, 128], F32)
            nc.vector.memset(ones32[:], 1.0)
            ones_bf = small.tile([1, 128], BF)
            nc.vector.tensor_copy(ones_bf[:], ones32[0:1, :])
            ones_r = small.tile([128, 64], F32R)
            nc.vector.tensor_copy(ones_r[:], ones32[:, 0:64])
            nc.vector.tensor_copy(
                v_sb[:, :, :, HD],
                ones32[:, 0 : LC * NH].rearrange("p (a b) -> p a b", a=LC),
            )

            # ---- input DMA: one descriptor-set per tensor (per-chunk
            # dispatch serialized ~0.6us each on the SP sequencer and
            # delayed compute start by ~6us). xt first (V-proj gates on
            # all of it), then wv/wk/wq/wo in consumption order; small
            # tensors go out on the DVE queue in parallel. ----
            for c in range(NC):
                nc.sync.dma_start(
                    xt_cs[c][:],
                    xt_e[:].rearrange("(c p) q -> p c q", p=128)[:, c, :],
                )
                nc.sync.dma_start(
                    wv_cs[c][:],
                    wvt_e[:].rearrange("(c p) d -> p c d", p=128)[:, c, :],
                )
            nc.sync.dma_start(
                wk_sb[:], wkt_e[:].rearrange("(c p) d -> p c d", p=128)
            )
            nc.sync.dma_start(
                wq_sb[:], wqt_e[:].rearrange("(c p) d -> p c d", p=128)
            )
            mask_sb = small.tile([128, LC], F32)
            nc.scalar.dma_start(mask_sb[:], mask_e[:].rearrange("(c p) -> p c", p=128))
            bq_sb = small.tile([128, NC], F32)
            nc.scalar.dma_start(bq_sb[:], bq_e[:].rearrange("(c p) -> p c", p=128))
            bk_sb = small.tile([128, NC], F32)
            nc.scalar.dma_start(bk_sb[:], bk_e[:].rearrange("(c p) -> p c", p=128))
            bv_sb = small.tile([128, NC], BF)
            nc.scalar.dma_start(bv_sb[:], bv_e[:].rearrange("(c p) -> p c", p=128))
            bo_sb = small.tile([1, H], BF)
            nc.scalar.dma_start(bo_sb[:], bo_e[None, :])
            nc.sync.dma_start(
                wo_sb[:], wot_e[:].rearrange("(c p) d -> p c d", p=128)
            )

            # =========== V projection: v[q, h, d] = x Wv^T ===========
            for lc in range(LC):
                ps = psA.tile([128, 1024], F32, tag="psA")
                for off, width in ((0, 512), (512, 256)):
                    for kc in range(NC):
                        nc.tensor.matmul(
                            ps[:, off : off + width],
                            xt_cs[kc][:, lc * 128 : lc * 128 + 128],
                            wv_cs[kc][:, off : off + width],
                            start=(kc == 0),
                            stop=(kc == NC - 1),
                        )
                nc.scalar.copy(
                    v_sb[:, lc, :, 0:HD],
                    ps[:, 0:H].rearrange("p (h d) -> p h d", d=HD),
                )

            # ---- K^T / Q^T projection emitters: out[d, q] = W x^T.
            # Emitted as 6-matmul half-projections (one qh each) so they
            # interleave into the attention pair loop as short PE bursts
            # the ACT exp stream can ride through. ----
            def emit_kproj_half(dc, qh):
                o = qh * 512
                ps = psA.tile([128, 512], F32, tag="psA")
                for kc in range(NC):
                    nc.tensor.matmul(
                        ps[:, 0:512],
                        wk_sb[:, kc, dc * 128 : dc * 128 + 128],
                        xt_cs[kc][:, o : o + 512],
                        start=(kc == 0),
                        stop=(kc == NC - 1),
                    )
                # split heads: even head -> rows 0:64 of slot 2dc, odd ->
                # rows 64:128 of slot 2dc+1 (rest stays zero-padded).
                # bk is added but cancels in softmax anyway; the add doubles
                # as the PSUM evacuation copy.
                nc.vector.tensor_scalar_add(
                    kt_sb[0:64, 2 * dc, o : o + 512], ps[0:64, 0:512],
                    bk_sb[0:64, dc : dc + 1],
                )
                nc.vector.tensor_scalar_add(
                    kt_sb[64:128, 2 * dc + 1, o : o + 512], ps[64:128, 0:512],
                    bk_sb[64:128, dc : dc + 1],
                )

            def emit_qproj_half(dc, qh):
                o = qh * 512
                ps = psA.tile([128, 512], F32, tag="psA")
                for kc in range(NC):
                    nc.tensor.matmul(
                        ps[:, 0:512],
                        wq_sb[:, kc, dc * 128 : dc * 128 + 128],
                        xt_cs[kc][:, o : o + 512],
                        start=(kc == 0),
                        stop=(kc == NC - 1),
                    )
                nc.vector.tensor_scalar_add(
                    qt_sb[:, dc, o : o + 512], ps[:, 0:512], bq_sb[:, dc : dc + 1]
                )

            for qh in range(2):
                emit_kproj_half(0, qh)
            for qh in range(2):
                emit_qproj_half(0, qh)

            # =========== attention: pair hp, with KQ proj of hp+1 ==========
            LOG = mybir.ActivationFunctionType.Ln

            def make_normalize(hp, ctxu_a, ctxu_b, last=False):
                rrows = [None, None]

                def chain_head(idx, ctxu, which):
                    # reciprocal chain — DMA/DVE/ACT only, NO PE
                    # instructions, so it can be emitted right after each
                    # evacuation and resolve its latency while the PE queue
                    # streams unrelated work. (PE-side consumers emitted
                    # here would head-of-line-block the in-order PE queue.)
                    if not last:
                        # SBUF->SBUF DMA the denominator row into [128,8]
                        # so the DVE reciprocal runs on all 128 lanes (a
                        # [1,1024] reciprocal is single-lane, 6.5us), cast
                        # to f32r cheaply while [128,8], DMA back to a row
                        rt = norm_pool.tile([128, 8], F32, tag="rt" + which)
                        nc.sync.dma_start(rt[:, :], ctxu[64:65, :])
                        rtr = norm_pool.tile([128, 8], F32, tag="rr" + which)
                        nc.vector.reciprocal(rtr[:], rt[:])
                        rtr_r = norm_pool.tile([128, 8], F32R, tag="rc" + which)
                        nc.vector.tensor_copy(rtr_r[:], rtr[:])
                        rrow = norm_pool.tile([1, 1024], F32R, tag="rw" + which)
                        nc.sync.dma_start(rrow[0:1, :], rtr_r[:, :])
                        rrows[idx] = rrow
                    else:
                        # last pair: the out projection gates on this, so
                        # use the short-latency ACT path per head, 1/d =
                        # exp(-ln d) straight from the evacuated row (ACT
                        # is idle by now; head a's chain runs during head
                        # b's PV sweep)
                        lnd = norm_pool.tile([1, 1024], F32, tag="ln" + which)
                        nc.scalar.activation(lnd[:], ctxu[64:65, :], LOG)
                        rrow = norm_pool.tile([1, 1024], F32R, tag="rw" + which)
                        nc.scalar.activation(rrow[:], lnd[:], EXP, scale=-1.0)
                        rrows[idx] = rrow

                def emit_finish(bc_pool):
                    for idx, (ctxu, which) in (
                        (0, (ctxu_a, "a")), (1, (ctxu_b, "b"))
                    ):
                        # broadcast 1/d over 64 partitions via a 1-row f32r
                        # matmul, then scale ctx^T. Even head multiplies on
                        # DVE reading bc straight from PSUM; odd head goes
                        # through Pool mid-kernel (parallel with DVE) but
                        # through DVE on the latency-critical last pair,
                        # then lifts to partitions 64:128 via DMA.
                        bc = bc_pool.tile(
                            [64, 1024], F32,
                            tag="psA" if bc_pool is psA else "psB",
                        )
                        for o in (0, 512):
                            nc.tensor.matmul(
                                bc[:, o : o + 512],
                                ones_r[0:1, 0:64],
                                rrows[idx][0:1, o : o + 512],
                                start=True,
                                stop=True,
                            )
                        if which == "a":
                            nc.vector.tensor_tensor(
                                ctxt_sb[0:64, hp, :], ctxu[0:64, :], bc[:],
                                mybir.AluOpType.mult,
                            )
                        else:
                            tmp_o = norm_pool.tile([64, 1024], BF, tag="tmp")
                            if last:
                                nc.vector.tensor_tensor(
                                    tmp_o[:], ctxu[0:64, :], bc[:],
                                    mybir.AluOpType.mult,
                                )
                            else:
                                bc_sb = norm_pool.tile(
                                    [64, 1024], F32, tag="bc" + which
                                )
                                nc.vector.tensor_copy(bc_sb[:], bc[:])
                                nc.gpsimd.tensor_tensor(
                                    tmp_o[:], ctxu[0:64, :], bc_sb[:],
                                    mybir.AluOpType.mult,
                                )
                            nc.sync.dma_start(ctxt_sb[64:128, hp, :], tmp_o[:])

                return chain_head, emit_finish

            # b' = bo + Wo @ bv (P rows sum to 1 after normalize, so bv
            # contributes bv @ Wo^T, a constant row added to every output
            # q). Emitted during pair NP-2 so its PE matmuls and the
            # single-lane DVE copy hide inside the attention stream
            # instead of gating the output projection.
            bbc_holder = [None]

            def emit_bprime():
                bps = psA.tile([1, 1024], F32, tag="psA")
                for off, width in ((0, 512), (512, 256)):
                    for c in range(NC):
                        nc.tensor.matmul(
                            bps[0:1, off : off + width],
                            bv_sb[:, c : c + 1],
                            wo_sb[:, c, off : off + width],
                            start=(c == 0),
                            stop=False,
                        )
                    nc.tensor.matmul(
                        bps[0:1, off : off + width],
                        ones_bf[0:1, 0:1],
                        bo_sb[0:1, off : off + width],
                        start=False,
                        stop=True,
                    )
                bexp_sb = small.tile([1, H], BF)
                nc.vector.tensor_copy(bexp_sb[:], bps[0:1, 0:H])
                bbc_ps = psA.tile([128, 1024], F32, tag="psA")
                for off, width in ((0, 512), (512, 256)):
                    nc.tensor.matmul(
                        bbc_ps[:, off : off + width],
                        ones_bf[0:1, 0:128],
                        bexp_sb[0:1, off : off + width],
                        start=True,
                        stop=True,
                    )
                bbc_sb = small.tile([128, H], F32)
                nc.vector.tensor_copy(bbc_sb[:], bbc_ps[:, 0:H])
                bbc_holder[0] = bbc_sb

            # Per pair, head a's PV accumulates during the kc loop and head
            # b's PV runs as a second sweep afterwards, so only ONE ctx
            # PSUM tile is ever live (psB bufs=1, 2 banks) — which buys
            # psA THREE [128,1024] ST slabs (6 banks). With only two, each
            # ST waited on the exp of the previous kc through a full
            # semaphore round-trip: the engines ran in lockstep with ~1us
            # of dead time per kc (87us total in the v5 trace).
            pending_norm = None
            for hp in range(NP):
                ha, hb = 2 * hp, 2 * hp + 1

                def emit_pv(kc, et, h, ctx):
                    first, last = kc == 0, kc == LC - 1
                    for qh in range(2):
                        o = qh * 512
                        nc.tensor.matmul(
                            ctx[0 : HD + 1, o : o + 512],
                            v_sb[:, kc, h, :],
                            et[:, o : o + 512],
                            start=first,
                            stop=last,
                        )

                # rows 0:64 = ctx^T, row 64 = softmax denominator
                ctx_a = psB.tile([128, 1024], F32, tag="psB")
                pv_q = []  # head-a PV at lag 2
                et_bs = []  # head-b et tiles, consumed in the second sweep
                for kc in range(LC):
                    if kc == 5 and pending_norm is not None:
                        pending_norm(psA)
                        pending_norm = None
                    st_a = psA.tile([128, 1024], F32, tag="psA")
                    st_b = psA.tile([128, 1024], F32, tag="psA")
                    for st, h in ((st_a, ha), (st_b, hb)):
                        for qh in range(2):
                            o = qh * 512
                            nc.tensor.matmul(
                                st[:, o : o + 512],
                                kt_sb[:, h, kc * 128 : kc * 128 + 128],
                                qt_sb[:, hp, o : o + 512],
                                start=True,
                                stop=True,
                            )
                    # P^T = exp(S^T/8 + mask_k)
                    et_a = eta_pool.tile([128, 1024], BF, tag="eta")
                    et_b = etb_pool.tile([128, 1024], BF, tag="etb")
                    nc.scalar.activation(
                        et_a[:], st_a[:], EXP,
                        bias=mask_sb[:, kc : kc + 1], scale=0.125,
                    )
                    nc.scalar.activation(
                        et_b[:], st_b[:], EXP,
                        bias=mask_sb[:, kc : kc + 1], scale=0.125,
                    )
                    pv_q.append((kc, et_a))
                    et_bs.append(et_b)
                    if len(pv_q) > 2:
                        kc_, et_ = pv_q.pop(0)
                        emit_pv(kc_, et_, ha, ctx_a)
                    # next pair's K/Q projection, spread as 6-matmul bursts
                    # emitted after this kc's PV so the PE always has
                    # slack-filling work between STs
                    if hp + 1 < NP:
                        if kc == 1:
                            emit_kproj_half(hp + 1, 0)
                        elif kc == 3:
                            emit_kproj_half(hp + 1, 1)
                        elif kc == 4:
                            emit_qproj_half(hp + 1, 0)
                while pv_q:
                    kc_, et_ = pv_q.pop(0)
                    emit_pv(kc_, et_, ha, ctx_a)
                # evacuate head a immediately: frees the single psB slab
                # for head b's sweep
                ctxu_a = norm_pool.tile([65, 1024], F32, tag="cua")
                nc.scalar.copy(ctxu_a[:], ctx_a[0:65, :])
                ctxu_b = norm_pool.tile([65, 1024], F32, tag="cub")
                chain_head, finish = make_normalize(
                    hp, ctxu_a, ctxu_b, last=(hp == NP - 1)
                )
                # head a's reciprocal chain resolves during head b's sweep
                chain_head(0, ctxu_a, "a")
                # fill the evacuation latency with the last projection burst
                if hp + 1 < NP:
                    emit_qproj_half(hp + 1, 1)
                ctx_b = psB.tile([128, 1024], F32, tag="psB")
                for kc in range(LC):
                    emit_pv(kc, et_bs[kc], hb, ctx_b)
                nc.vector.tensor_copy(ctxu_b[:], ctx_b[0:65, :])
                chain_head(1, ctxu_b, "b")
                if hp == NP - 2:
                    emit_bprime()
                pending_norm = finish
            last_norm = pending_norm
            pending_norm = None
            bbc_sb = bbc_holder[0]

            # ---- output projection: out[q, o] = ctx_norm Wo^T + b'.
            # Chunks 0..4 of the first two groups are emitted BEFORE the
            # last pair's normalize matmuls so the PE streams them while
            # that pair's reciprocal chain resolves; chunk 5 (written by
            # the last normalize) closes each accumulation group. ----
            def outproj_front(ps, lc):
                for off, width in ((0, 512), (512, 256)):
                    for c in range(NC - 1):
                        nc.tensor.matmul(
                            ps[:, off : off + width],
                            ctxt_sb[:, c, lc * 128 : lc * 128 + 128],
                            wo_sb[:, c, off : off + width],
                            start=(c == 0),
                            stop=False,
                        )

            def outproj_close(ps, lc):
                c = NC - 1
                for off, width in ((0, 512), (512, 256)):
                    nc.tensor.matmul(
                        ps[:, off : off + width],
                        ctxt_sb[:, c, lc * 128 : lc * 128 + 128],
                        wo_sb[:, c, off : off + width],
                        start=False,
                        stop=True,
                    )
                o_sb = out_pool.tile([128, H], F32, tag="outp")
                nc.vector.tensor_tensor(
                    o_sb[:], ps[:, 0:H], bbc_sb[:], mybir.AluOpType.add
                )
                nc.sync.dma_start(out_e[lc * 128 : lc * 128 + 128, :], o_sb[:])

            ps0 = psA.tile([128, 1024], F32, tag="psA")
            outproj_front(ps0, 0)
            ps1 = psA.tile([128, 1024], F32, tag="psA")
            outproj_front(ps1, 1)
            last_norm(psB)  # bc tiles from the (now free) psB slab
            outproj_close(ps0, 0)
            outproj_close(ps1, 1)
            for lc in range(2, LC):
                ps = psA.tile([128, 1024], F32, tag="psA")
                outproj_front(ps, lc)
                outproj_close(ps, lc)

    nc.finalize()
    nc.m = get_hw_module(nc.m)
    return nc


_NC_CACHE = {}


def _get_nc(compute_rounded: bool = True):
    if compute_rounded not in _NC_CACHE:
        _NC_CACHE[compute_rounded] = build_bass(compute_rounded)
    return _NC_CACHE[compute_rounded]


def make_in_maps(inputs):
    f32 = lambda a: np.ascontiguousarray(np.asarray(a, dtype=np.float32))  # noqa: E731
    bf = lambda a: np.ascontiguousarray(  # noqa: E731
        np.asarray(a, dtype=np.float32).astype(ml_dtypes.bfloat16)
    )
    hs = np.asarray(inputs["hidden_states"], dtype=np.float32)
    mask = f32(inputs["attention_mask"]).reshape(B, L)
    shared = {
        "wqt": bf(np.asarray(inputs["Wq"]).T),
        "wkt": bf(np.asarray(inputs["Wk"]).T),
        "wvt": bf(np.asarray(inputs["Wv"]).T),
        "wot": bf(np.asarray(inputs["Wo"]).T),
        "bq": f32(inputs["bq"]),
        "bk": f32(inputs["bk"]),
        "bv": bf(inputs["bv"]),
        "bo": bf(inputs["bo"]),
    }
    return [
        {"xt": bf(hs[b].T), "mask": mask[b], **shared}
        for b in range(B)
    ]


def run_spmd(inputs, trace=False, compute_rounded=True):
    nc = _get_nc(compute_rounded)
    res = run_bass_kernel_spmd(nc, make_in_maps(inputs), list(range(B)), trace=trace)
    out = np.stack([res.results[b]["out"] for b in range(B)]).astype(np.float32)
    return out, res


def kernel(**inputs) -> np.ndarray:
    out, _ = run_spmd(inputs, trace=False)
    return out
